# revision 4
# baseline (speedup 1.0000x reference)
"""Trainium2 Bass kernel for nn_Classifier (segment mean-pool + tiny MLP head).

Pipeline (matches the jax reference):
  pooled[g] = mean of features over nodes with batch id g   (2048 graphs)
  out = LeakyReLU(LayerNorm(pooled @ W1 + b1)) @ W2 + b2    -> [2048, 1]

Sharding: batch ids are sorted, so nodes split across the 8 cores at
segment-block boundaries — core i owns graphs [256i, 256i+256) and exactly
the nodes belonging to them; the host concatenates the per-core outputs.

v3 design (v2 measured 124 us; stream was already at the ~358 GB/s HBM
roofline, the rest was startup latency, a late head, and fat one-hots):
 - features stream in bf16 (end-to-end rel err ~2e-3, 10x under the gate).
 - nodes are laid out in 32-segment *windows*, each window padded to a whole
   number of 128-node subtiles with caps shared across all 8 cores, so every
   subtile maps to one window known at build time. One-hots are then only
   [128 nodes, 32 segs]: is_equal work drops 4x, and the matmul writes its
   32-row band via tile_position col-tiling (window w -> array columns
   32*(w%4), PSUM rows 32*(w%4)).
 - one-hots are built in batches of 64 subtiles: one scalar-engine broadcast
   of the window-relative seg ids + one vector tensor_tensor is_equal (both
   bf16; the old fp32r destination forced a 1.5-2.6us/op DVE slow path).
 - per-region epilogue (pooled, transpose, MLP head, output DMA) is emitted
   right after that region's chunks, so region 0's head hides under region
   1's streaming and only region 1's tiny head remains in the tail.
 - GPSIMD is never used; DMA runs on the two HWDGE queues (sync + scalar).
"""

from contextlib import ExitStack

import numpy as np
import ml_dtypes

import concourse.bass as bass
import concourse.mybir as mybir
import concourse.tile as tile
from concourse.bass_utils import run_bass_kernel_spmd

# ---------------------------------------------------------------------------
# Workaround: this walrus build rejects instructions carrying more than one
# semaphore wait ("Too many sync wait commands"), but Tile's semaphore
# assignment freely attaches several. After the TileContext has lowered the
# program, split any excess waits onto same-engine nops inserted right before
# the instruction (semantics are identical: all waits are monotonic and must
# hold before the instruction issues).
_MAX_WAITS = 1


def _split_excess_waits(nc: "bass.Bass", max_waits: int = _MAX_WAITS) -> None:
    ctr = 0
    for f in nc.m.functions:
        for b in f.blocks:
            out = []
            for inst in b.instructions:
                si = inst.sync_info
                waits = list(si.on_wait) if (si is not None and si.on_wait) else []
                if len(waits) > max_waits:
                    keep = waits[-max_waits:]
                    extra = waits[:-max_waits]
                    # On the PE queue the carrier must be a DRAIN: silicon
                    # promotes waitless LDWEIGHTS past in-flight work, so a
                    # plain nop's wait can be bypassed (walrus attaches a
                    # matmul's waits to its LDWEIGHTS — stripping them onto a
                    # nop re-opens that race). A drain fully serializes.
                    is_pe = inst.engine == mybir.EngineType.PE
                    for i in range(0, len(extra), max_waits):
                        ctr += 1
                        if is_pe:
                            nop = mybir.InstDrain(
                                name=f"waitsplit_drain_{ctr}", ins=[], outs=[],
                                engine=inst.engine,
                            )
                        else:
                            nop = mybir.InstNoOp(
                                name=f"waitsplit_nop_{ctr}", ins=[], outs=[],
                                engine=inst.engine,
                            )
                        nop.sync_info = mybir.SyncInfo(
                            on_wait=extra[i : i + max_waits], on_update=[]
                        )
                        nc.register_instruction(nop)
                        out.append(nop)
                    inst.sync_info = mybir.SyncInfo(
                        on_wait=keep, on_update=list(si.on_update or [])
                    )
                out.append(inst)
            b.instructions = out
# ---------------------------------------------------------------------------

N_CORES = 8
NUM_GRAPHS = 2048
SEGS_PER_CORE = NUM_GRAPHS // N_CORES  # 256
D = 256
K_SUB = 8  # 128-node sub-tiles per DMA chunk (chunk = 1024 nodes, 512 KB bf16)
CHUNK = 128 * K_SUB
W = 32  # segs per window (= one-hot width = tile_position column group)
BCOLS = 64  # subtile columns per one-hot generation batch
LN_EPS = 1e-5
NEG_SLOPE = 0.01

_F32 = mybir.dt.float32
_BF16 = mybir.dt.bfloat16
_ALU = mybir.AluOpType

# Test/debug hooks: set PROFILE=True before calling kernel() to request an
# NTFF trace; the BassKernelResults lands in LAST_RESULT.
PROFILE = False
PROFILE_DIR = None
LAST_RESULT = None


def _build_program(R: int, w_lo: list[int]) -> bass.Bass:
    """R: chunks per region. w_lo: first subtile of each of the 4 windows in a
    region (shared by both regions and all cores by construction)."""
    S = R * K_SUB  # subtiles per region
    C = 2 * R  # chunks per core
    n_nodes = C * CHUNK
    w_hi = w_lo[1:] + [S]

    def slot_of(t: int) -> int:
        for j in range(3, -1, -1):
            if t >= w_lo[j]:
                return j
        return 0

    nc = bass.Bass("TRN2", debug=False)
    feat = nc.dram_tensor("feat", [n_nodes, D], _BF16, kind="ExternalInput").ap()
    segT = nc.dram_tensor("segT", [128, 2 * S], _BF16, kind="ExternalInput").ap()
    iotab_d = nc.dram_tensor("iotab", [128, BCOLS * W], _BF16, kind="ExternalInput").ap()
    ident_d = nc.dram_tensor("ident", [128, 128], _F32, kind="ExternalInput").ap()
    w1aug_d = nc.dram_tensor("w1aug", [D + 1, 128], _F32, kind="ExternalInput").ap()
    pvec_d = nc.dram_tensor("pvec", [1, 385], _F32, kind="ExternalInput").ap()
    recip_d = nc.dram_tensor("recip", [128, 2], _F32, kind="ExternalInput").ap()
    out_d = nc.dram_tensor("out", [2, 128], _F32, kind="ExternalOutput").ap()

    with tile.TileContext(nc) as tc, ExitStack() as ctx:
        cpool = ctx.enter_context(tc.tile_pool(name="consts", bufs=1))
        fpool = ctx.enter_context(tc.tile_pool(name="feat", bufs=6))
        opool = ctx.enter_context(tc.tile_pool(name="oh", bufs=3))
        acc = ctx.enter_context(tc.tile_pool(name="acc", bufs=1, space="PSUM"))
        ppool = ctx.enter_context(tc.tile_pool(name="pw", bufs=2, space="PSUM"))
        spool = ctx.enter_context(tc.tile_pool(name="small", bufs=2))

        # stream-critical consts first (tiny; on the sync queue ahead of data)
        segT_t = cpool.tile([128, 2 * S], _BF16, tag="segT")
        nc.sync.dma_start(out=segT_t[:], in_=segT[:])
        iotab_t = cpool.tile([128, BCOLS, W], _BF16, tag="iotab")
        nc.sync.dma_start(
            out=iotab_t[:], in_=iotab_d[:].rearrange("p (c s) -> p c s", s=W)
        )
        # head consts (needed only ~half-way in; scalar queue)
        ident_t = cpool.tile([128, 128], _F32, tag="ident")
        nc.scalar.dma_start(out=ident_t[:], in_=ident_d[:])
        w1a = cpool.tile([128, 128], _F32, tag="w1a")
        nc.scalar.dma_start(out=w1a[:], in_=w1aug_d[0:128, :])
        w1b = cpool.tile([128, 128], _F32, tag="w1b")
        nc.scalar.dma_start(out=w1b[:], in_=w1aug_d[128:256, :])
        w1c = cpool.tile([1, 128], _F32, tag="w1c")
        nc.scalar.dma_start(out=w1c[:], in_=w1aug_d[256:257, :])
        pv = cpool.tile([1, 385], _F32, tag="pv")
        nc.scalar.dma_start(out=pv[:], in_=pvec_d[:])
        recip_t = cpool.tile([128, 2], _F32, tag="recip")
        nc.scalar.dma_start(out=recip_t[:], in_=recip_d[:])
        ones_row = cpool.tile([1, 256], _F32, tag="ones")
        nc.vector.memset(ones_row[:], 1.0)
        epsc = cpool.tile([128, 1], _F32, tag="epsc")
        nc.vector.memset(epsc[:], LN_EPS)

        # broadcast [gamma | beta | W2 | b2] to all 128 partitions
        bc_ps = ppool.tile([128, 385], _F32, tag="bc")
        nc.tensor.matmul(
            out=bc_ps[:], lhsT=ones_row[:, 0:128], rhs=pv[:], start=True, stop=True
        )
        bc = cpool.tile([128, 385], _F32, tag="bcs")
        nc.scalar.copy(bc[:], bc_ps[:])

        sums = [acc.tile([128, D], _F32, tag=f"sum{r}", name=f"sum{r}") for r in range(2)]
        ptT = [spool.tile([128, 256], _F32, tag=f"ptT{fb}", name=f"ptT{fb}") for fb in range(2)]

        def emit_epilogue(r: int) -> None:
            # pooled = sums * recip(counts); transpose; tiny MLP head; out DMA
            pooled = spool.tile([128, 256], _F32, tag="pooled")
            nc.vector.tensor_scalar(
                out=pooled[:], in0=sums[r][:], scalar1=recip_t[:, r : r + 1],
                scalar2=None, op0=_ALU.mult,
            )
            for fb in range(2):
                tp = ppool.tile([128, 128], _F32, tag="tp")
                nc.tensor.transpose(
                    out=tp[:], in_=pooled[:, fb * 128 : (fb + 1) * 128],
                    identity=ident_t[:],
                )
                nc.scalar.copy(ptT[fb][:, r * 128 : (r + 1) * 128], tp[:])

            m = r
            msl = slice(m * 128, (m + 1) * 128)
            h_ps = ppool.tile([128, 128], _F32, tag="h")
            nc.tensor.matmul(
                out=h_ps[:], lhsT=ptT[0][:, msl], rhs=w1a[:], start=True, stop=False
            )
            nc.tensor.matmul(
                out=h_ps[:], lhsT=ptT[1][:, msl], rhs=w1b[:], start=False, stop=False
            )
            nc.tensor.matmul(
                out=h_ps[:], lhsT=ones_row[:, msl], rhs=w1c[:], start=False, stop=True
            )
            musum = spool.tile([128, 1], _F32, tag="musum")
            nc.vector.tensor_reduce(
                out=musum[:], in_=h_ps[:], axis=mybir.AxisListType.X, op=_ALU.add
            )
            mu = spool.tile([128, 1], _F32, tag="mu")
            nc.vector.tensor_scalar(
                out=mu[:], in0=musum[:], scalar1=1.0 / 128, scalar2=None, op0=_ALU.mult
            )
            hc = spool.tile([128, 128], _F32, tag="hc")
            nc.vector.tensor_scalar(
                out=hc[:], in0=h_ps[:], scalar1=mu[:], scalar2=None, op0=_ALU.subtract
            )
            sq = spool.tile([128, 128], _F32, tag="sq")
            ssq = spool.tile([128, 1], _F32, tag="ssq")
            nc.vector.scalar_tensor_tensor(
                out=sq[:], in0=hc[:], scalar=1.0, in1=hc[:],
                op0=_ALU.mult, op1=_ALU.mult, accum_out=ssq[:],
            )
            std = spool.tile([128, 1], _F32, tag="std")
            nc.scalar.activation(
                std[:], ssq[:], mybir.ActivationFunctionType.Sqrt,
                bias=epsc[:], scale=1.0 / 128,
            )
            rstd = spool.tile([128, 1], _F32, tag="rstd")
            nc.vector.reciprocal(rstd[:], std[:])
            y = spool.tile([128, 128], _F32, tag="y")
            nc.vector.scalar_tensor_tensor(
                out=y[:], in0=hc[:], scalar=rstd[:], in1=bc[:, 0:128],
                op0=_ALU.mult, op1=_ALU.mult,
            )
            y2 = spool.tile([128, 128], _F32, tag="y2")
            nc.vector.tensor_tensor(out=y2[:], in0=y[:], in1=bc[:, 128:256],
                                    op=_ALU.add)
            yl = spool.tile([128, 128], _F32, tag="yl")
            nc.vector.scalar_tensor_tensor(
                out=yl[:], in0=y2[:], scalar=NEG_SLOPE, in1=y2[:],
                op0=_ALU.mult, op1=_ALU.max,
            )
            prod = spool.tile([128, 128], _F32, tag="prod")
            oc = spool.tile([128, 1], _F32, tag="oc")
            nc.vector.scalar_tensor_tensor(
                out=prod[:], in0=yl[:], scalar=1.0, in1=bc[:, 256:384],
                op0=_ALU.mult, op1=_ALU.mult, accum_out=oc[:],
            )
            ofin = spool.tile([128, 1], _F32, tag="ofin")
            nc.vector.tensor_scalar(
                out=ofin[:], in0=oc[:], scalar1=bc[:, 384:385], scalar2=None,
                op0=_ALU.add,
            )
            nc.sync.dma_start(out=out_d[m, :], in_=ofin[:])

        # ---- main stream: per-segment sums via 32-wide bf16 one-hots ----
        oh = None
        oh_base = 0
        for r in range(2):
            for c in range(R):
                chunk = r * R + c
                ft = fpool.tile([128, K_SUB, D], _BF16, tag="ft")
                src = feat[chunk * CHUNK : (chunk + 1) * CHUNK, :].rearrange(
                    "(p k) f -> p k f", p=128
                )
                dma_eng = nc.sync if chunk % 2 == 0 else nc.scalar
                dma_eng.dma_start(out=ft[:], in_=src)
                for k in range(K_SUB):
                    g = chunk * K_SUB + k  # global subtile column
                    if g % BCOLS == 0:
                        oh_base = g
                        nb = min(BCOLS, 2 * S - g)
                        segB = opool.tile([128, nb, W], _BF16, tag="segB")
                        nc.scalar.copy(
                            segB[:],
                            segT_t[:, g : g + nb]
                            .unsqueeze(2)
                            .broadcast_to([128, nb, W]),
                        )
                        oh = opool.tile([128, nb, W], _BF16, tag="oh")
                        nc.vector.tensor_tensor(
                            out=oh[:], in0=segB[:], in1=iotab_t[:, 0:nb, :],
                            op=_ALU.is_equal,
                        )
                    t = g - r * S  # subtile within region
                    j = slot_of(t)
                    nc.tensor.matmul(
                        out=sums[r][j * W : (j + 1) * W, :],
                        lhsT=oh[:, g - oh_base, :],
                        rhs=ft[:, k, :],
                        start=(t == w_lo[j]),
                        stop=(t == w_hi[j] - 1),
                        tile_position=(0, j * W),
                    )
            emit_epilogue(r)

    _split_excess_waits(nc)
    return nc


def _prep_inputs(features, batch):
    """Window-aligned sharding: core i owns segs [256i, 256i+256), split into
    8 windows of 32 segs; each window padded to whole 128-node subtiles with
    per-window-slot caps shared across cores (so one program fits all)."""
    feats = np.asarray(features, dtype=np.float32)
    seg = np.asarray(batch).astype(np.int64)
    counts = np.bincount(seg, minlength=NUM_GRAPHS)
    bnd = np.zeros(NUM_GRAPHS + 1, np.int64)
    bnd[1:] = np.cumsum(counts)

    # nodes per (core, window-slot): 8 cores x 8 slots of 32 segs
    wcnt = counts.reshape(N_CORES, 8, W).sum(-1)  # [core, slot]
    wsub = (wcnt + 127) // 128  # subtiles needed
    cap = wsub.max(axis=0)  # [8] shared caps
    # one slot layout for both regions: per-position max across the regions
    cap2 = np.maximum(cap[0:4], cap[4:8])
    reg_subs = int(cap2.sum())
    R = (reg_subs + K_SUB - 1) // K_SUB  # chunks per region
    S = R * K_SUB  # subtiles per region
    w_lo = [0, int(cap2[0]), int(cap2[0] + cap2[1]), int(cap2[0] + cap2[1] + cap2[2])]

    ncap = 2 * S * 128
    stream_f = np.zeros((N_CORES, ncap, D), ml_dtypes.bfloat16)
    stream_s = np.full((N_CORES, ncap), -1.0, np.float32)
    for i in range(N_CORES):
        for r in range(2):
            for j in range(4):
                sl = 4 * r + j
                glo = bnd[(2 * i + r) * 128 + j * W]
                ghi = bnd[(2 * i + r) * 128 + (j + 1) * W]
                m = int(ghi - glo)
                off = (r * S + w_lo[j]) * 128
                stream_f[i, off : off + m] = feats[glo:ghi].astype(ml_dtypes.bfloat16)
                stream_s[i, off : off + m] = (seg[glo:ghi] - ((2 * i + r) * 128 + j * W)).astype(np.float32)
    # subtile-major -> chunk/partition-major: node (t*128 + p) lands at flat
    # slot (t//8)*1024 + p*8 + (t%8), i.e. [C,8,128,D] -> [C,128,8,D]
    C = 2 * R
    feat_aug = np.ascontiguousarray(
        stream_f.reshape(N_CORES, C, K_SUB, 128, D).transpose(0, 1, 3, 2, 4)
    ).reshape(N_CORES, ncap, D)
    segT = np.ascontiguousarray(
        stream_s.reshape(N_CORES, 2 * S, 128).transpose(0, 2, 1).astype(ml_dtypes.bfloat16)
    )
    recip = (1.0 / np.maximum(counts.astype(np.float64), 1.0)).astype(np.float32)
    recip = np.ascontiguousarray(recip.reshape(N_CORES, 2, 128).transpose(0, 2, 1))
    return feat_aug, segT, recip, R, w_lo


def kernel(features, batch, W1, b1, gamma, beta, W2, b2):
    feat_aug, segT, recip, R, w_lo = _prep_inputs(features, batch)

    iotab = np.tile(
        np.arange(W, dtype=np.float32)[None, None, :], (128, BCOLS, 1)
    ).reshape(128, -1).astype(ml_dtypes.bfloat16)
    ident = np.eye(128, dtype=np.float32)
    w1aug = np.concatenate(
        [np.asarray(W1, np.float32), np.asarray(b1, np.float32)[None, :]], axis=0
    )
    pvec = np.concatenate(
        [
            np.asarray(gamma, np.float32).ravel(),
            np.asarray(beta, np.float32).ravel(),
            np.asarray(W2, np.float32).ravel(),
            np.asarray(b2, np.float32).ravel(),
        ]
    )[None, :]

    nc = _build_program(R, w_lo)
    in_maps = [
        {
            "feat": feat_aug[i],
            "segT": segT[i],
            "iotab": iotab,
            "ident": ident,
            "w1aug": w1aug,
            "pvec": pvec,
            "recip": recip[i],
        }
        for i in range(N_CORES)
    ]
    res = run_bass_kernel_spmd(
        nc, in_maps, list(range(N_CORES)), trace=PROFILE, tmpdir=PROFILE_DIR
    )
    global LAST_RESULT
    LAST_RESULT = res
    out = np.concatenate(
        [res.results[i]["out"].reshape(SEGS_PER_CORE) for i in range(N_CORES)]
    )
    return out.reshape(NUM_GRAPHS, 1).astype(np.float32)
